# revision 15
# baseline (speedup 1.0000x reference)
"""Trainium2 Bass kernel for 3x3 VALID conv (nn_BreakupConv).

x [16,64,128,128] f32, weights [128,64,9] f32 -> out [16,128,126,126] f32.

Strategy ("pair" scheme):
- Data-parallel: 2 images per NeuronCore (8 cores).
- Image A occupies SBUF partitions 0-63, image B partitions 64-127 (both
  as [C_in=64, H*W] fp16). Each 3x3 tap is issued as TWO K=64 matmuls --
  one per image -- on disjoint PE row groups (tile_position auto-derived
  from base_partition 0 / 64). Row-tiled matmuls execute concurrently
  (measured dstart ~4ns on TRN2), so the 9 taps cost ~9 concurrent spans
  per PAIR of output tiles = 4.5 PE columns per output position: the
  theoretical minimum for K=64 contraction on a 128-row array, with no
  input duplication (the classic scheme needs a shifted copy of x to
  fill 128 partitions, doubling input DMA bytes).
- Output is written fp16 (host converts back to f32): halves output HBM
  traffic. rel err ~4e-4 total vs fp32 reference (fp16 matmul ~2.7e-4 +
  fp16 store rounding ~2.8e-4).
- PSUM drains split across engines: DVE drains image A tiles, ACT
  (scalar) drains image B tiles, so neither engine is on the critical
  path. Drains convert f32 PSUM -> fp16 SBUF; DMA groups 4 row-tiles
  (16 output rows) per transfer to amortize DGE overhead, with the final
  group split [2,1,1] to shorten the drain->DMA->receipt tail.
- Single-shot startup: the weights DMA ships tap-0's block first, and 56
  tiny (N=64) warmup matmuls on a zeroed tile fill the ~3.5us DMA-startup
  window so the HAM clock gate opens before real tiles start.

Per-core HBM traffic: 4.2 MB in + 8.1 MB out (+0.3 MB weights) ~= 12.6 MB
vs 24.8 MB for the classic f32-out scheme; at ~358 GB/s/core this is the
DMA roofline ~35us, balanced against ~30us of PE time ("ridge" regime).
"""

import os
import numpy as np

os.environ.setdefault("BASS_NEVER_TRACE", "1")

B, C_IN, H, W = 16, 64, 128, 128
C_OUT, HO, WO = 128, 126, 126
N_CORES = 8
IMGS_PER_CORE = B // N_CORES
HW = H * W           # 16384
ROWS_PER_TILE = 4    # output rows per PSUM tile (4*126 = 504 <= 512)
N_TAPS = 9

SCHEME = os.environ.get("CONV_SCHEME", "pair")
MM_DTYPE = os.environ.get("CONV_MM_DTYPE", "float16")
OUT_DTYPE = os.environ.get("CONV_OUT_DTYPE", "float16")
N_CHUNKS = int(os.environ.get("CONV_CHUNKS", "16"))
TILES_PER_OUT = int(os.environ.get("CONV_TILES_PER_OUT", "4"))
PSUM_BUFS = int(os.environ.get("CONV_PSUM_BUFS", "3"))
OUT_BUFS = int(os.environ.get("CONV_OUT_BUFS", "3"))
N_WARM = int(os.environ.get("CONV_WARMUP", "56"))
WARM_N = int(os.environ.get("CONV_WARM_N", "64"))  # free dim of warm MMs
B_DRAIN = os.environ.get("CONV_B_DRAIN", "scalar")
EXTRA_TAPS = int(os.environ.get("CONV_EXTRA_TAPS", "0"))  # PE-load probe
HP_DMA = os.environ.get("CONV_HP_DMA", "0") == "1"
OUT_ENG = os.environ.get("CONV_OUT_ENG", "sync")
TAIL_SPLIT = os.environ.get("CONV_TAIL_SPLIT", "1") == "1"

# classic-scheme slots (kept for A/B benchmarking): (rhs offset (ky,kx),
# upper tap, lower tap); lower SBUF half holds x shifted +1 element.
SLOTS_CLASSIC = [
    ((0, 0), 0, 1),
    ((1, 0), 3, 4),
    ((2, 0), 6, 7),
    ((0, 2), 2, None),
    ((1, 2), 5, None),
    ((2, 2), 8, None),
]

_CACHE = {}


def _build_program(reps=1, scheme=None, xp_bufs=2):
    import concourse.bacc as bacc
    import concourse.mybir as mybir
    from concourse.tile import TileContext

    scheme = scheme or SCHEME
    dt = getattr(mybir.dt, MM_DTYPE)
    out_dt = getattr(mybir.dt, OUT_DTYPE)
    f32 = mybir.dt.float32

    nc = bacc.Bacc(None, target_bir_lowering=False)
    n_tiles = (HO + ROWS_PER_TILE - 1) // ROWS_PER_TILE

    n_taps_eff = N_TAPS + EXTRA_TAPS
    if scheme == "pair":
        x2_d = nc.dram_tensor("x2", [IMGS_PER_CORE, C_IN, HW], dt,
                              kind="ExternalInput")
        w_d = nc.dram_tensor("wmm", [128, n_taps_eff * 128], dt,
                             kind="ExternalInput")
        out_d = nc.dram_tensor("out2", [IMGS_PER_CORE, C_OUT, HO * WO],
                               out_dt, kind="ExternalOutput")
        with TileContext(nc) as tc:
            with (
                tc.tile_pool(name="xp", bufs=xp_bufs) as xp,
                tc.tile_pool(name="wp", bufs=1) as wp,
                tc.tile_pool(name="pp", bufs=PSUM_BUFS, space="PSUM") as pp,
                tc.tile_pool(name="op", bufs=OUT_BUFS) as op,
            ):
                w_sb = wp.tile([128, n_taps_eff * 128], dt)
                # tap-0 block first so the first matmul isn't gated on the
                # whole weight transfer.
                nc.sync.dma_start(out=w_sb[:, 0:128], in_=w_d[:, 0:128])
                nc.sync.dma_start(out=w_sb[:, 128:], in_=w_d[:, 128:])
                if N_WARM:
                    # Tiny-N PE warmup filling the initial DMA-wait window:
                    # keeps the HAM activity monitor busy so the clock gate
                    # opens (1.2 -> 2.4 GHz) before real tiles start, at
                    # ~56ns per MM and zero added latency. Reads a zeroed
                    # tile (no DMA dependency -> starts immediately).
                    warm_w = wp.tile([128, 128], dt)
                    nc.any.memset(warm_w[:], 0.0)
                    warm_ps = pp.tile([128, WARM_N], f32, name="warm_ps",
                                      tag="warm", bufs=1)
                    for wi in range(N_WARM):
                        nc.tensor.matmul(
                            warm_ps[:],
                            warm_w[:],
                            warm_w[:, 0:WARM_N],
                            start=(wi == 0),
                            stop=(wi == N_WARM - 1),
                        )
                import contextlib
                out_eng = getattr(nc, OUT_ENG)
                for _rep in range(reps):
                    x_sb = xp.tile([128, HW], dt)
                    csz = HW // N_CHUNKS
                    hp = tc.high_priority() if HP_DMA else contextlib.nullcontext()
                    with hp:
                        for ci in range(N_CHUNKS):
                            sl = slice(ci * csz, (ci + 1) * csz)
                            nc.sync.dma_start(out=x_sb[0:C_IN, sl],
                                              in_=x2_d[0, :, sl])
                            nc.sync.dma_start(out=x_sb[C_IN:128, sl],
                                              in_=x2_d[1, :, sl])
                    xv = x_sb[:].rearrange("p (h w) -> p h w", h=H)

                    # group sizes: TILES_PER_OUT each, but split the final
                    # group fine-grained so the tail (last drain -> last DMA
                    # -> completion receipt) chains over a small transfer.
                    groups = []
                    rem = n_tiles
                    while rem > TILES_PER_OUT:
                        groups.append(TILES_PER_OUT)
                        rem -= TILES_PER_OUT
                    if TAIL_SPLIT and rem > 1:
                        groups += [rem - 2] if rem > 2 else []
                        groups += [1, 1]
                    else:
                        groups.append(rem)
                    t = 0
                    for tg in groups:
                        y0 = t * ROWS_PER_TILE
                        rg = min(HO - y0, tg * ROWS_PER_TILE)
                        ot_a = op.tile([128, rg * WO], out_dt, name="ot_a",
                                       tag="ot_a")
                        ot_b = op.tile([128, rg * WO], out_dt, name="ot_b",
                                       tag="ot_b")
                        for ti in range(tg):
                            y = y0 + ti * ROWS_PER_TILE
                            r = min(ROWS_PER_TILE, HO - y)
                            ps_a = pp.tile([128, r * WO], f32, name="ps_a",
                                           tag="ps_a")
                            ps_b = pp.tile([128, r * WO], f32, name="ps_b",
                                           tag="ps_b")
                            for s in range(n_taps_eff):
                                ky, kx = divmod(min(s, N_TAPS - 1), 3)
                                blk = slice(s * 128, (s + 1) * 128)
                                win = (slice(y + ky, y + ky + r),
                                       slice(kx, kx + WO))
                                nc.tensor.matmul(
                                    ps_a[:],
                                    w_sb[0:C_IN, blk],
                                    xv[0:C_IN, win[0], win[1]],
                                    start=(s == 0),
                                    stop=(s == n_taps_eff - 1),
                                )
                                nc.tensor.matmul(
                                    ps_b[:],
                                    w_sb[C_IN:128, blk],
                                    xv[C_IN:128, win[0], win[1]],
                                    start=(s == 0),
                                    stop=(s == n_taps_eff - 1),
                                )
                            oc = ti * ROWS_PER_TILE * WO
                            nc.vector.tensor_copy(
                                ot_a[:, oc:oc + r * WO], ps_a[:])
                            if B_DRAIN == "scalar":
                                nc.scalar.copy(
                                    ot_b[:, oc:oc + r * WO], ps_b[:])
                            else:
                                nc.vector.tensor_copy(
                                    ot_b[:, oc:oc + r * WO], ps_b[:])
                        out_eng.dma_start(
                            out=out_d[0, :, y0 * WO:(y0 + rg) * WO],
                            in_=ot_a[:])
                        out_eng.dma_start(
                            out=out_d[1, :, y0 * WO:(y0 + rg) * WO],
                            in_=ot_b[:])
                        t += tg
    elif scheme == "classic":
        # Baseline scheme: host sends [128, HW] per image (upper 64 =
        # channels, lower 64 = shifted +1), K=128 matmuls, 6 slots.
        x2_d = nc.dram_tensor("x2", [IMGS_PER_CORE, 128, HW], dt,
                              kind="ExternalInput")
        w_d = nc.dram_tensor("wmm", [128, len(SLOTS_CLASSIC) * 128], dt,
                             kind="ExternalInput")
        out_d = nc.dram_tensor("out2", [IMGS_PER_CORE, C_OUT, HO * WO],
                               out_dt, kind="ExternalOutput")
        with TileContext(nc) as tc:
            with (
                tc.tile_pool(name="xp", bufs=xp_bufs) as xp,
                tc.tile_pool(name="wp", bufs=1) as wp,
                tc.tile_pool(name="pp", bufs=2 * PSUM_BUFS, space="PSUM") as pp,
                tc.tile_pool(name="op", bufs=2 * OUT_BUFS) as op,
            ):
                w_sb = wp.tile([128, len(SLOTS_CLASSIC) * 128], dt)
                nc.sync.dma_start(out=w_sb[:], in_=w_d[:])
                if N_WARM:
                    warm_ps = pp.tile([128, 504], f32, name="warm_ps",
                                      tag="warm", bufs=1)
                    for wi in range(N_WARM):
                        nc.tensor.matmul(
                            warm_ps[:], w_sb[:, 0:128], w_sb[:, 0:504],
                            start=(wi == 0), stop=(wi == N_WARM - 1))
                for _rep in range(reps):
                    for img in range(IMGS_PER_CORE):
                        x_sb = xp.tile([128, HW], dt)
                        csz = HW // N_CHUNKS
                        for ci in range(N_CHUNKS):
                            sl = slice(ci * csz, (ci + 1) * csz)
                            nc.sync.dma_start(out=x_sb[:, sl],
                                              in_=x2_d[img, :, sl])
                        xv = x_sb[:].rearrange("p (h w) -> p h w", h=H)
                        t = 0
                        while t < n_tiles:
                            tg = min(TILES_PER_OUT, n_tiles - t)
                            y0 = t * ROWS_PER_TILE
                            rg = min(HO - y0, tg * ROWS_PER_TILE)
                            ot = op.tile([128, rg * WO], out_dt, name="ot",
                                         tag="ot")
                            for ti in range(tg):
                                y = y0 + ti * ROWS_PER_TILE
                                r = min(ROWS_PER_TILE, HO - y)
                                ps = pp.tile([128, r * WO], f32, name="ps",
                                             tag="ps")
                                for s, ((ky, kx), _ta, _tb) in enumerate(
                                        SLOTS_CLASSIC):
                                    nc.tensor.matmul(
                                        ps[:],
                                        w_sb[:, s * 128:(s + 1) * 128],
                                        xv[:, y + ky:y + ky + r, kx:kx + WO],
                                        start=(s == 0),
                                        stop=(s == len(SLOTS_CLASSIC) - 1),
                                    )
                                oc = ti * ROWS_PER_TILE * WO
                                eng = (nc.vector.tensor_copy
                                       if ti % 2 == 0 or B_DRAIN != "scalar"
                                       else nc.scalar.copy)
                                eng(ot[:, oc:oc + r * WO], ps[:])
                            nc.sync.dma_start(
                                out=out_d[img, :, y0 * WO:(y0 + rg) * WO],
                                in_=ot[:])
                            t += tg
    else:
        raise ValueError(f"unknown scheme {scheme}")
    nc.compile()
    return nc


def _make_runner(nc):
    """Build a reusable jitted SPMD callable for `nc` over 8 cores.

    Returns (run, meta): run(list_of_global_np_inputs) -> list of global
    np outputs with shape (N_CORES*dim0, ...). Inputs are device_put once
    per call; no donation (outputs fully written by the kernel).
    """
    import jax
    import concourse.mybir as mybir
    from concourse import bass2jax
    from jax.experimental.shard_map import shard_map
    from jax.sharding import Mesh, NamedSharding, PartitionSpec

    bass2jax.install_neuronx_cc_hook()

    partition_name = (
        nc.partition_id_tensor.name if nc.partition_id_tensor is not None else None
    )
    in_names, out_names, out_avals, zero_outs = [], [], [], []
    for alloc in nc.m.functions[0].allocations:
        if not isinstance(alloc, mybir.MemoryLocationSet):
            continue
        name = alloc.memorylocations[0].name
        if alloc.kind == "ExternalInput":
            if name != partition_name:
                in_names.append(name)
        elif alloc.kind == "ExternalOutput":
            out_names.append(name)
            shape = tuple(alloc.tensor_shape)
            dtype = mybir.dt.np(alloc.dtype)
            out_avals.append(jax.core.ShapedArray(shape, dtype))
            zero_outs.append(np.zeros(shape, dtype))
    n_params = len(in_names)
    all_in_names = list(in_names) + list(out_names)
    if partition_name is not None:
        all_in_names.append(partition_name)

    def _body(*args):
        operands = list(args)
        if partition_name is not None:
            operands.append(bass2jax.partition_id_tensor())
        outs = bass2jax._bass_exec_p.bind(
            *operands,
            out_avals=tuple(out_avals),
            in_names=tuple(all_in_names),
            out_names=tuple(out_names),
            lowering_input_output_aliases=(),
            sim_require_finite=True,
            sim_require_nnan=True,
            nc=nc,
        )
        return tuple(outs)

    devices = jax.devices()[:N_CORES]
    mesh = Mesh(np.asarray(devices), ("core",))
    spec = PartitionSpec("core")
    n_args = n_params + len(out_names)
    sharded = jax.jit(
        shard_map(
            _body,
            mesh=mesh,
            in_specs=(spec,) * n_args,
            out_specs=(spec,) * len(out_names),
            check_rep=False,
        ),
        keep_unused=True,
    )
    sharding = NamedSharding(mesh, spec)
    zeros_dev = [
        jax.device_put(np.zeros((N_CORES * z.shape[0], *z.shape[1:]), z.dtype),
                       sharding)
        for z in zero_outs
    ]

    def run(global_inputs, device_inputs=None):
        if device_inputs is None:
            device_inputs = [jax.device_put(g, sharding) for g in global_inputs]
        outs = sharded(*device_inputs, *zeros_dev)
        jax.block_until_ready(outs)
        return outs

    meta = {
        "sharding": sharding,
        "out_avals": out_avals,
        "out_names": out_names,
        "jax": jax,
        "sharded": sharded,
        "zeros_dev": zeros_dev,
    }
    return run, meta


def get_runner(reps=1):
    key = ("runner", SCHEME, reps)
    if key not in _CACHE:
        nc = _build_program(reps)
        _CACHE[key] = _make_runner(nc)
    return _CACHE[key]


def _np_mm_dtype():
    return {"float16": np.float16, "bfloat16": None}.get(MM_DTYPE, np.float32)


def prep_inputs(x, weights):
    """Host-side shard prep: returns global (concat over cores) inputs."""
    npdt = _np_mm_dtype()
    if npdt is None:
        import ml_dtypes
        npdt = ml_dtypes.bfloat16
    x = np.asarray(x)
    weights = np.asarray(weights, dtype=np.float32)

    base = np.asarray(x, dtype=np.float32).astype(npdt).reshape(B, C_IN, HW)
    if SCHEME == "pair":
        x2_global = base
        wmm = np.zeros((128, (N_TAPS + EXTRA_TAPS) * 128), npdt)
        for s in range(N_TAPS):
            wt = weights[:, :, s].T.astype(npdt)  # [C_in, C_out]
            wmm[0:C_IN, s * 128:(s + 1) * 128] = wt
            wmm[C_IN:128, s * 128:(s + 1) * 128] = wt
    else:
        x2_global = np.empty((B, 2 * C_IN, HW), npdt)
        x2_global[:, :C_IN, :] = base
        x2_global[:, C_IN:, :HW - 1] = base[:, :, 1:]
        x2_global[:, C_IN:, HW - 1] = 0
        wmm = np.zeros((128, len(SLOTS_CLASSIC) * 128), npdt)
        for s, (_off, ta, tb) in enumerate(SLOTS_CLASSIC):
            if ta is not None:
                wmm[0:64, s * 128:(s + 1) * 128] = weights[:, :, ta].T
            if tb is not None:
                wmm[64:128, s * 128:(s + 1) * 128] = weights[:, :, tb].T
    wmm_global = np.tile(wmm, (N_CORES, 1))
    return [x2_global, wmm_global]


def kernel(x, weights):
    run, _meta = get_runner()
    outs = run(prep_inputs(x, weights))
    out_g = np.asarray(outs[0]).astype(np.float32)  # [8*2, 128, HO*WO]
    return out_g.reshape(B, C_OUT, HO, WO)


# revision 17
# speedup vs baseline: 2.4946x; 2.4946x over previous
"""Trainium2 Bass kernel for 3x3 VALID conv (nn_BreakupConv).

x [16,64,128,128] f32, weights [128,64,9] f32 -> out [16,128,126,126] f32.

Strategy ("pair" scheme):
- Data-parallel: 2 images per NeuronCore (8 cores).
- Image A occupies SBUF partitions 0-63, image B partitions 64-127 (both
  as [C_in=64, H*W] fp16). Each 3x3 tap is issued as TWO K=64 matmuls --
  one per image -- on disjoint PE row groups (tile_position auto-derived
  from base_partition 0 / 64). Row-tiled matmuls execute concurrently
  (measured dstart ~4ns on TRN2), so the 9 taps cost ~9 concurrent spans
  per PAIR of output tiles = 4.5 PE columns per output position: the
  theoretical minimum for K=64 contraction on a 128-row array, with no
  input duplication (the classic scheme needs a shifted copy of x to
  fill 128 partitions, doubling input DMA bytes).
- Output is written fp16 (host converts back to f32): halves output HBM
  traffic. rel err ~4e-4 total vs fp32 reference (fp16 matmul ~2.7e-4 +
  fp16 store rounding ~2.8e-4).
- PSUM drains split across engines: DVE drains image A tiles, ACT
  (scalar) drains image B tiles, so neither engine is on the critical
  path. Drains convert f32 PSUM -> fp16 SBUF; DMA groups 4 row-tiles
  (16 output rows) per transfer to amortize DGE overhead, with the final
  group split [2,1,1] to shorten the drain->DMA->receipt tail.
- Single-shot startup: the weights DMA ships tap-0's block first, and 56
  tiny (N=64) warmup matmuls on a zeroed tile fill the ~3.5us DMA-startup
  window so the HAM clock gate opens before real tiles start.

Per-core HBM traffic: 4.2 MB in + 8.1 MB out (+0.3 MB weights) ~= 12.6 MB
vs 24.8 MB for the classic f32-out scheme; at ~358 GB/s/core this is the
DMA roofline ~35us, balanced against ~30us of PE time ("ridge" regime).
"""

import os
import numpy as np

os.environ.setdefault("BASS_NEVER_TRACE", "1")

B, C_IN, H, W = 16, 64, 128, 128
C_OUT, HO, WO = 128, 126, 126
N_CORES = 8
IMGS_PER_CORE = B // N_CORES
HW = H * W           # 16384
ROWS_PER_TILE = 4    # output rows per PSUM tile (4*126 = 504 <= 512)
N_TAPS = 9

SCHEME = os.environ.get("CONV_SCHEME", "pair")
MM_DTYPE = os.environ.get("CONV_MM_DTYPE", "float16")
OUT_DTYPE = os.environ.get("CONV_OUT_DTYPE", "float16")
N_CHUNKS = int(os.environ.get("CONV_CHUNKS", "16"))
# "auto": two 1024-elem chunks (fast pipeline start, hidden behind the
# warmup window) then 2048-elem chunks -> 9 input DMAs per image instead
# of 16, cutting serial HWDGE descriptor-generation load ~25%.
CHUNK_PLAN = os.environ.get("CONV_CHUNK_PLAN", "auto")
TILES_PER_OUT = int(os.environ.get("CONV_TILES_PER_OUT", "4"))
PSUM_BUFS = int(os.environ.get("CONV_PSUM_BUFS", "3"))
OUT_BUFS = int(os.environ.get("CONV_OUT_BUFS", "3"))
N_WARM = int(os.environ.get("CONV_WARMUP", "56"))
WARM_N = int(os.environ.get("CONV_WARM_N", "64"))  # free dim of warm MMs
B_DRAIN = os.environ.get("CONV_B_DRAIN", "scalar")
EXTRA_TAPS = int(os.environ.get("CONV_EXTRA_TAPS", "0"))  # PE-load probe
HP_DMA = os.environ.get("CONV_HP_DMA", "0") == "1"
OUT_ENG = os.environ.get("CONV_OUT_ENG", "sync")
TAIL_SPLIT = os.environ.get("CONV_TAIL_SPLIT", "1") == "1"

# classic-scheme slots (kept for A/B benchmarking): (rhs offset (ky,kx),
# upper tap, lower tap); lower SBUF half holds x shifted +1 element.
SLOTS_CLASSIC = [
    ((0, 0), 0, 1),
    ((1, 0), 3, 4),
    ((2, 0), 6, 7),
    ((0, 2), 2, None),
    ((1, 2), 5, None),
    ((2, 2), 8, None),
]

_CACHE = {}


def _build_program(reps=1, scheme=None, xp_bufs=2):
    import concourse.bacc as bacc
    import concourse.mybir as mybir
    from concourse.tile import TileContext

    scheme = scheme or SCHEME
    dt = getattr(mybir.dt, MM_DTYPE)
    out_dt = getattr(mybir.dt, OUT_DTYPE)
    f32 = mybir.dt.float32

    nc = bacc.Bacc(None, target_bir_lowering=False)
    n_tiles = (HO + ROWS_PER_TILE - 1) // ROWS_PER_TILE

    n_taps_eff = N_TAPS + EXTRA_TAPS
    if scheme == "pair":
        x2_d = nc.dram_tensor("x2", [IMGS_PER_CORE, C_IN, HW], dt,
                              kind="ExternalInput")
        w_d = nc.dram_tensor("wmm", [128, n_taps_eff * 128], dt,
                             kind="ExternalInput")
        out_d = nc.dram_tensor("out2", [IMGS_PER_CORE, C_OUT, HO * WO],
                               out_dt, kind="ExternalOutput")
        with TileContext(nc) as tc:
            with (
                tc.tile_pool(name="xp", bufs=xp_bufs) as xp,
                tc.tile_pool(name="wp", bufs=1) as wp,
                tc.tile_pool(name="pp", bufs=PSUM_BUFS, space="PSUM") as pp,
                tc.tile_pool(name="op", bufs=OUT_BUFS) as op,
            ):
                w_sb = wp.tile([128, n_taps_eff * 128], dt)
                # tap-0 block first so the first matmul isn't gated on the
                # whole weight transfer.
                nc.sync.dma_start(out=w_sb[:, 0:128], in_=w_d[:, 0:128])
                nc.sync.dma_start(out=w_sb[:, 128:], in_=w_d[:, 128:])
                if N_WARM:
                    # Tiny-N PE warmup filling the initial DMA-wait window:
                    # keeps the HAM activity monitor busy so the clock gate
                    # opens (1.2 -> 2.4 GHz) before real tiles start, at
                    # ~56ns per MM and zero added latency. Reads a zeroed
                    # tile (no DMA dependency -> starts immediately).
                    warm_w = wp.tile([128, 128], dt)
                    nc.any.memset(warm_w[:], 0.0)
                    warm_ps = pp.tile([128, WARM_N], f32, name="warm_ps",
                                      tag="warm", bufs=1)
                    for wi in range(N_WARM):
                        nc.tensor.matmul(
                            warm_ps[:],
                            warm_w[:],
                            warm_w[:, 0:WARM_N],
                            start=(wi == 0),
                            stop=(wi == N_WARM - 1),
                        )
                import contextlib
                out_eng = getattr(nc, OUT_ENG)
                if CHUNK_PLAN == "auto":
                    chunk_sizes = [1024, 1024] + [2048] * 7
                else:
                    chunk_sizes = [HW // N_CHUNKS] * N_CHUNKS
                assert sum(chunk_sizes) == HW, chunk_sizes
                for _rep in range(reps):
                    x_sb = xp.tile([128, HW], dt)
                    hp = tc.high_priority() if HP_DMA else contextlib.nullcontext()
                    with hp:
                        lo = 0
                        for csz in chunk_sizes:
                            sl = slice(lo, lo + csz)
                            nc.sync.dma_start(out=x_sb[0:C_IN, sl],
                                              in_=x2_d[0, :, sl])
                            nc.sync.dma_start(out=x_sb[C_IN:128, sl],
                                              in_=x2_d[1, :, sl])
                            lo += csz
                    xv = x_sb[:].rearrange("p (h w) -> p h w", h=H)

                    # group sizes: TILES_PER_OUT each, but split the final
                    # group fine-grained so the tail (last drain -> last DMA
                    # -> completion receipt) chains over a small transfer.
                    groups = []
                    rem = n_tiles
                    while rem > TILES_PER_OUT:
                        groups.append(TILES_PER_OUT)
                        rem -= TILES_PER_OUT
                    if TAIL_SPLIT and rem > 1:
                        groups += [rem - 2] if rem > 2 else []
                        groups += [1, 1]
                    else:
                        groups.append(rem)
                    t = 0
                    for tg in groups:
                        y0 = t * ROWS_PER_TILE
                        rg = min(HO - y0, tg * ROWS_PER_TILE)
                        ot_a = op.tile([128, rg * WO], out_dt, name="ot_a",
                                       tag="ot_a")
                        ot_b = op.tile([128, rg * WO], out_dt, name="ot_b",
                                       tag="ot_b")
                        for ti in range(tg):
                            y = y0 + ti * ROWS_PER_TILE
                            r = min(ROWS_PER_TILE, HO - y)
                            ps_a = pp.tile([128, r * WO], f32, name="ps_a",
                                           tag="ps_a")
                            ps_b = pp.tile([128, r * WO], f32, name="ps_b",
                                           tag="ps_b")
                            for s in range(n_taps_eff):
                                ky, kx = divmod(min(s, N_TAPS - 1), 3)
                                blk = slice(s * 128, (s + 1) * 128)
                                win = (slice(y + ky, y + ky + r),
                                       slice(kx, kx + WO))
                                nc.tensor.matmul(
                                    ps_a[:],
                                    w_sb[0:C_IN, blk],
                                    xv[0:C_IN, win[0], win[1]],
                                    start=(s == 0),
                                    stop=(s == n_taps_eff - 1),
                                )
                                nc.tensor.matmul(
                                    ps_b[:],
                                    w_sb[C_IN:128, blk],
                                    xv[C_IN:128, win[0], win[1]],
                                    start=(s == 0),
                                    stop=(s == n_taps_eff - 1),
                                )
                            oc = ti * ROWS_PER_TILE * WO
                            nc.vector.tensor_copy(
                                ot_a[:, oc:oc + r * WO], ps_a[:])
                            if B_DRAIN == "scalar":
                                nc.scalar.copy(
                                    ot_b[:, oc:oc + r * WO], ps_b[:])
                            else:
                                nc.vector.tensor_copy(
                                    ot_b[:, oc:oc + r * WO], ps_b[:])
                        out_eng.dma_start(
                            out=out_d[0, :, y0 * WO:(y0 + rg) * WO],
                            in_=ot_a[:])
                        out_eng.dma_start(
                            out=out_d[1, :, y0 * WO:(y0 + rg) * WO],
                            in_=ot_b[:])
                        t += tg
    elif scheme == "classic":
        # Baseline scheme: host sends [128, HW] per image (upper 64 =
        # channels, lower 64 = shifted +1), K=128 matmuls, 6 slots.
        x2_d = nc.dram_tensor("x2", [IMGS_PER_CORE, 128, HW], dt,
                              kind="ExternalInput")
        w_d = nc.dram_tensor("wmm", [128, len(SLOTS_CLASSIC) * 128], dt,
                             kind="ExternalInput")
        out_d = nc.dram_tensor("out2", [IMGS_PER_CORE, C_OUT, HO * WO],
                               out_dt, kind="ExternalOutput")
        with TileContext(nc) as tc:
            with (
                tc.tile_pool(name="xp", bufs=xp_bufs) as xp,
                tc.tile_pool(name="wp", bufs=1) as wp,
                tc.tile_pool(name="pp", bufs=2 * PSUM_BUFS, space="PSUM") as pp,
                tc.tile_pool(name="op", bufs=2 * OUT_BUFS) as op,
            ):
                w_sb = wp.tile([128, len(SLOTS_CLASSIC) * 128], dt)
                nc.sync.dma_start(out=w_sb[:], in_=w_d[:])
                if N_WARM:
                    warm_ps = pp.tile([128, 504], f32, name="warm_ps",
                                      tag="warm", bufs=1)
                    for wi in range(N_WARM):
                        nc.tensor.matmul(
                            warm_ps[:], w_sb[:, 0:128], w_sb[:, 0:504],
                            start=(wi == 0), stop=(wi == N_WARM - 1))
                for _rep in range(reps):
                    for img in range(IMGS_PER_CORE):
                        x_sb = xp.tile([128, HW], dt)
                        csz = HW // N_CHUNKS
                        for ci in range(N_CHUNKS):
                            sl = slice(ci * csz, (ci + 1) * csz)
                            nc.sync.dma_start(out=x_sb[:, sl],
                                              in_=x2_d[img, :, sl])
                        xv = x_sb[:].rearrange("p (h w) -> p h w", h=H)
                        t = 0
                        while t < n_tiles:
                            tg = min(TILES_PER_OUT, n_tiles - t)
                            y0 = t * ROWS_PER_TILE
                            rg = min(HO - y0, tg * ROWS_PER_TILE)
                            ot = op.tile([128, rg * WO], out_dt, name="ot",
                                         tag="ot")
                            for ti in range(tg):
                                y = y0 + ti * ROWS_PER_TILE
                                r = min(ROWS_PER_TILE, HO - y)
                                ps = pp.tile([128, r * WO], f32, name="ps",
                                             tag="ps")
                                for s, ((ky, kx), _ta, _tb) in enumerate(
                                        SLOTS_CLASSIC):
                                    nc.tensor.matmul(
                                        ps[:],
                                        w_sb[:, s * 128:(s + 1) * 128],
                                        xv[:, y + ky:y + ky + r, kx:kx + WO],
                                        start=(s == 0),
                                        stop=(s == len(SLOTS_CLASSIC) - 1),
                                    )
                                oc = ti * ROWS_PER_TILE * WO
                                eng = (nc.vector.tensor_copy
                                       if ti % 2 == 0 or B_DRAIN != "scalar"
                                       else nc.scalar.copy)
                                eng(ot[:, oc:oc + r * WO], ps[:])
                            nc.sync.dma_start(
                                out=out_d[img, :, y0 * WO:(y0 + rg) * WO],
                                in_=ot[:])
                            t += tg
    else:
        raise ValueError(f"unknown scheme {scheme}")
    nc.compile()
    return nc


def _make_runner(nc):
    """Build a reusable jitted SPMD callable for `nc` over 8 cores.

    Returns (run, meta): run(list_of_global_np_inputs) -> list of global
    np outputs with shape (N_CORES*dim0, ...). Inputs are device_put once
    per call; no donation (outputs fully written by the kernel).
    """
    import jax
    import concourse.mybir as mybir
    from concourse import bass2jax
    from jax.experimental.shard_map import shard_map
    from jax.sharding import Mesh, NamedSharding, PartitionSpec

    bass2jax.install_neuronx_cc_hook()

    partition_name = (
        nc.partition_id_tensor.name if nc.partition_id_tensor is not None else None
    )
    in_names, out_names, out_avals, zero_outs = [], [], [], []
    for alloc in nc.m.functions[0].allocations:
        if not isinstance(alloc, mybir.MemoryLocationSet):
            continue
        name = alloc.memorylocations[0].name
        if alloc.kind == "ExternalInput":
            if name != partition_name:
                in_names.append(name)
        elif alloc.kind == "ExternalOutput":
            out_names.append(name)
            shape = tuple(alloc.tensor_shape)
            dtype = mybir.dt.np(alloc.dtype)
            out_avals.append(jax.core.ShapedArray(shape, dtype))
            zero_outs.append(np.zeros(shape, dtype))
    n_params = len(in_names)
    all_in_names = list(in_names) + list(out_names)
    if partition_name is not None:
        all_in_names.append(partition_name)

    def _body(*args):
        operands = list(args)
        if partition_name is not None:
            operands.append(bass2jax.partition_id_tensor())
        outs = bass2jax._bass_exec_p.bind(
            *operands,
            out_avals=tuple(out_avals),
            in_names=tuple(all_in_names),
            out_names=tuple(out_names),
            lowering_input_output_aliases=(),
            sim_require_finite=True,
            sim_require_nnan=True,
            nc=nc,
        )
        return tuple(outs)

    devices = jax.devices()[:N_CORES]
    mesh = Mesh(np.asarray(devices), ("core",))
    spec = PartitionSpec("core")
    n_args = n_params + len(out_names)
    sharded = jax.jit(
        shard_map(
            _body,
            mesh=mesh,
            in_specs=(spec,) * n_args,
            out_specs=(spec,) * len(out_names),
            check_rep=False,
        ),
        keep_unused=True,
    )
    sharding = NamedSharding(mesh, spec)
    zeros_dev = [
        jax.device_put(np.zeros((N_CORES * z.shape[0], *z.shape[1:]), z.dtype),
                       sharding)
        for z in zero_outs
    ]

    def run(global_inputs, device_inputs=None):
        if device_inputs is None:
            device_inputs = [jax.device_put(g, sharding) for g in global_inputs]
        outs = sharded(*device_inputs, *zeros_dev)
        jax.block_until_ready(outs)
        return outs

    meta = {
        "sharding": sharding,
        "out_avals": out_avals,
        "out_names": out_names,
        "jax": jax,
        "sharded": sharded,
        "zeros_dev": zeros_dev,
    }
    return run, meta


def get_runner(reps=1):
    key = ("runner", SCHEME, reps)
    if key not in _CACHE:
        nc = _build_program(reps)
        _CACHE[key] = _make_runner(nc)
    return _CACHE[key]


def _np_mm_dtype():
    return {"float16": np.float16, "bfloat16": None}.get(MM_DTYPE, np.float32)


def prep_inputs(x, weights):
    """Host-side shard prep: returns global (concat over cores) inputs."""
    npdt = _np_mm_dtype()
    if npdt is None:
        import ml_dtypes
        npdt = ml_dtypes.bfloat16
    x = np.asarray(x)
    weights = np.asarray(weights, dtype=np.float32)

    base = np.asarray(x, dtype=np.float32).astype(npdt).reshape(B, C_IN, HW)
    if SCHEME == "pair":
        x2_global = base
        wmm = np.zeros((128, (N_TAPS + EXTRA_TAPS) * 128), npdt)
        for s in range(N_TAPS):
            wt = weights[:, :, s].T.astype(npdt)  # [C_in, C_out]
            wmm[0:C_IN, s * 128:(s + 1) * 128] = wt
            wmm[C_IN:128, s * 128:(s + 1) * 128] = wt
    else:
        x2_global = np.empty((B, 2 * C_IN, HW), npdt)
        x2_global[:, :C_IN, :] = base
        x2_global[:, C_IN:, :HW - 1] = base[:, :, 1:]
        x2_global[:, C_IN:, HW - 1] = 0
        wmm = np.zeros((128, len(SLOTS_CLASSIC) * 128), npdt)
        for s, (_off, ta, tb) in enumerate(SLOTS_CLASSIC):
            if ta is not None:
                wmm[0:64, s * 128:(s + 1) * 128] = weights[:, :, ta].T
            if tb is not None:
                wmm[64:128, s * 128:(s + 1) * 128] = weights[:, :, tb].T
    wmm_global = np.tile(wmm, (N_CORES, 1))
    return [x2_global, wmm_global]


def kernel(x, weights):
    run, _meta = get_runner()
    outs = run(prep_inputs(x, weights))
    out_g = np.asarray(outs[0]).astype(np.float32)  # [8*2, 128, HO*WO]
    return out_g.reshape(B, C_OUT, HO, WO)
